# revision 1
# baseline (speedup 1.0000x reference)
"""TRN2 Bass kernel for a GPT transformer block (B=4, T=2048, C=1024, H=16, dff=4096).

Sharding: 8 NeuronCores, core c = (batch b=c//2, parity p=c%2). Each core owns
the interleaved 128-row sequence chunks {2j+p : j<8} of batch b (this balances
causal-attention work between the two cores of a batch), computes full-sequence
k/v for its batch itself (no collectives), and produces its 1024 own rows of the
output. All 8 cores run ONE identical SPMD program; per-core behavior differs
only through data: the host permutes each core's x so its own chunks come first
(own-prefix order) and supplies per-parity causal masks.

On-chip layout: activations are feature-major ("transposed", [feature, row]) so
every GEMM contracts along the partition dim and outputs stay feature-major.
Attention computes scores transposed [ki, qi]; softmax needs no max-subtraction
(|scores| is small for this distribution); the denominator comes free from a
ones-column augmented onto V; causality = multiply exp tiles by {0,1} masks.

Precision: float32r (fp32 with 11-bit mantissa, full PE rate) for all GEMMs
except fc2 (bf16; the gelu output is cast there anyway). Weights are pre-rounded
to the f32r grid on the host, pre-arranged into partition-major contiguous slabs
(SWDGE descriptor count scales with discontiguity), and DMA'd directly.
"""
import numpy as np
import ml_dtypes

import concourse.bacc as bacc
import concourse.mybir as mybir
import concourse.tile as tile
from concourse.bass_utils import run_bass_kernel_spmd
from concourse.masks import make_identity

F32 = mybir.dt.float32
F32R = mybir.dt.float32r
BF16 = mybir.dt.bfloat16
AF = mybir.ActivationFunctionType
ALU = mybir.AluOpType
AX = mybir.AxisListType

B, T, C, H, HD, DFF = 4, 2048, 1024, 16, 64, 4096
NCH = T // 128          # 16 sequence chunks of 128
NOWN = 8                # own row chunks per core
R = NOWN * 128          # 1024 own rows
EPS = 1e-5


def _f32r_round(x):
    b = np.ascontiguousarray(x, dtype=np.float32).view(np.uint32).astype(np.uint64)
    b = ((b + 0x800) & 0xFFFFF000).astype(np.uint32)
    return b.view(np.float32)


def _slab(w, n_in_ch, slab_cols):
    """[Cin, Cout] -> [n_slabs, 128, n_in_ch, slab_cols] contiguous slabs."""
    cin, cout = w.shape
    assert cin == n_in_ch * 128 and cout % slab_cols == 0
    b = w.reshape(n_in_ch, 128, cout // slab_cols, slab_cols)
    return np.ascontiguousarray(b.transpose(2, 1, 0, 3))


def _layernorm_tiles(nc, statpool, x_ap, out_ap, tag, eps_ap):
    """LN stats (DVE bn_stats) + apply (ACT) for one [128, C] row tile."""
    bns = statpool.tile([128, 2, 6], F32, tag=f"{tag}bns")
    nc.vector.bn_stats(bns[:, 0, :], x_ap[:, 0:512])
    nc.vector.bn_stats(bns[:, 1, :], x_ap[:, 512:1024])
    ag = statpool.tile([128, 2], F32, tag=f"{tag}ag")
    nc.vector.bn_aggr(ag[:], bns[:])
    sig = statpool.tile([128, 1], F32, tag=f"{tag}sig")
    nc.scalar.activation(sig[:], ag[:, 1:2], AF.Sqrt, bias=eps_ap)
    rsig = statpool.tile([128, 1], F32, tag=f"{tag}rsig")
    nc.vector.reciprocal(rsig[:], sig[:])
    nmr = statpool.tile([128, 1], F32, tag=f"{tag}nmr")
    nc.vector.scalar_tensor_tensor(nmr[:], ag[:, 0:1], -1.0, rsig[:],
                                   op0=ALU.mult, op1=ALU.mult)
    nc.scalar.activation(out_ap, x_ap, AF.Identity, bias=nmr[:], scale=rsig[:])


def build_program(debug=False):
    nc = bacc.Bacc(None, target_bir_lowering=False, enable_partition_id=False)

    x_in = nc.declare_dram_parameter("x", [T, C], F32, isOutput=False)
    wqk_in = nc.declare_dram_parameter("wqk", [8, 128, 8, 256], F32R, isOutput=False)
    bqk_in = nc.declare_dram_parameter("bqk", [2 * C], F32, isOutput=False)
    wv_in = nc.declare_dram_parameter("wv", [128, 8, C], F32R, isOutput=False)
    bv_in = nc.declare_dram_parameter("bv", [C], F32R, isOutput=False)
    wproj_in = nc.declare_dram_parameter("wproj", [4, 128, 8, 256], F32R, isOutput=False)
    bproj_in = nc.declare_dram_parameter("bproj", [C], F32, isOutput=False)
    wfc_in = nc.declare_dram_parameter("wfc", [16, 128, 8, 256], F32R, isOutput=False)
    bfc_in = nc.declare_dram_parameter("bfc", [DFF], F32, isOutput=False)
    wfc2_in = nc.declare_dram_parameter("wfc2", [8, 128, 32, 128], F32R, isOutput=False)
    bfc2_in = nc.declare_dram_parameter("bfc2", [C], F32, isOutput=False)
    masks_in = nc.declare_dram_parameter("masks", [128, 2, 8, 512], F32R, isOutput=False)
    out_d = nc.declare_dram_parameter("out", [R, C], F32, isOutput=True)

    dbg = {}
    if debug:
        for nm, shp, dt_ in [("dbg_hT", [128, 8, T], F32R), ("dbg_qT", [128, 8, R], F32R),
                             ("dbg_yT", [128, 8, R], F32R),
                             ("dbg_h2T", [128, 8, R], F32R), ("dbg_gT", [128, 32, R], F32R)]:
            dbg[nm] = nc.declare_dram_parameter(nm, shp, dt_, isOutput=True)

    kT_d = nc.dram_tensor("kT_scratch", [8, 128, T], F32R)
    x1_d = nc.dram_tensor("x1_scratch", [8, 128, C], F32)
    vn_d = nc.dram_tensor("vn_scratch", [NCH, 128, C], F32R)

    x_r = x_in[:].rearrange("(t p) c -> t p c", p=128)

    with tile.TileContext(nc) as tc:
        with (
            tc.tile_pool(name="persist", bufs=1) as persist,
            tc.tile_pool(name="biasp", bufs=1) as biasp,
        ):
            identity = persist.tile([128, 128], F32)
            make_identity(nc, identity[:])
            ones_f = persist.tile([128, 128], F32)
            nc.gpsimd.memset(ones_f[:], 1.0)
            ones_r = persist.tile([1, 128], F32R)
            nc.scalar.copy(ones_r[:], ones_f[0:1, :])
            eps_t = persist.tile([128, 1], F32)
            nc.gpsimd.memset(eps_t[:], EPS)
            bqk_sb = biasp.tile([128, 16], F32)
            nc.gpsimd.dma_start(out=bqk_sb[:], in_=bqk_in[:].rearrange("(m p) -> p m", p=128))
            bv_sb = biasp.tile([1, C], F32R)
            nc.gpsimd.dma_start(out=bv_sb[:], in_=bv_in[:].rearrange("(o c) -> o c", o=1))
            bproj_sb = biasp.tile([128, 8], F32)
            nc.gpsimd.dma_start(out=bproj_sb[:], in_=bproj_in[:].rearrange("(m p) -> p m", p=128))
            bfc_sb = biasp.tile([128, 32], F32)
            nc.gpsimd.dma_start(out=bfc_sb[:], in_=bfc_in[:].rearrange("(m p) -> p m", p=128))
            bfc2_sb = biasp.tile([128, 8], F32)
            nc.gpsimd.dma_start(out=bfc2_sb[:], in_=bfc2_in[:].rearrange("(m p) -> p m", p=128))

            wv_pool = tc.alloc_tile_pool(name="wv_pool", bufs=1)
            wv_sb = wv_pool.tile([128, 8, C], F32R)
            nc.sync.dma_start(out=wv_sb[:], in_=wv_in[:])

            # ---- Stage 1: LN1 over all T (permuted) rows -> hT [C, T] f32r
            hT_pool = tc.alloc_tile_pool(name="hT_pool", bufs=1)
            hT = hT_pool.tile([128, 8, T], F32R)
            with (
                tc.tile_pool(name="s1w", bufs=3) as s1w,
                tc.tile_pool(name="s1s", bufs=3) as s1s,
                tc.tile_pool(name="s1p", bufs=4, space="PSUM") as s1p,
            ):
                for rt2 in range(NCH // 2):
                    xt2 = s1w.tile([128, 2, C], F32, tag="xt")
                    nc.scalar.dma_start(
                        out=xt2[:], in_=x_r[2 * rt2:2 * rt2 + 2].rearrange("t p c -> p t c"))
                    for sub in range(2):
                        rt = 2 * rt2 + sub
                        ht = s1w.tile([128, C], F32, tag="ht")
                        _layernorm_tiles(nc, s1s, xt2[:, sub, :], ht[:], "s1", eps_t[:])
                        for ci in range(8):
                            pt = s1p.tile([128, 128], F32, tag="pt")
                            nc.tensor.transpose(pt[:], ht[:, ci * 128:(ci + 1) * 128],
                                                identity[:])
                            nc.vector.tensor_copy(hT[:, ci, rt * 128:(rt + 1) * 128], pt[:])

            # ---- Stage 2: qkv GEMMs (hT still alive)
            qT_pool = tc.alloc_tile_pool(name="qT_pool", bufs=1, side="right")
            qT = qT_pool.tile([128, 8, R], F32R)

            if True:
                with (
                    tc.tile_pool(name="s2w", bufs=3) as s2w,
                    tc.tile_pool(name="s2ev", bufs=2) as s2ev,
                    tc.tile_pool(name="s2p", bufs=4, space="PSUM") as s2p,
                ):
                    for rt in range(NCH):
                        vb = s2ev.tile([128, C], F32R, tag="vb")
                        for n in range(2):
                            acc = s2p.tile([128, 512], F32, tag="vacc")
                            for ci in range(8):
                                nc.tensor.matmul(acc[:], hT[:, ci, rt * 128:(rt + 1) * 128],
                                                 wv_sb[:, ci, n * 512:(n + 1) * 512],
                                                 start=(ci == 0), stop=False)
                            nc.tensor.matmul(acc[:], ones_r[:, :],
                                             bv_sb[:, n * 512:(n + 1) * 512],
                                             start=False, stop=True)
                            nc.scalar.activation(vb[:, n * 512:(n + 1) * 512], acc[:],
                                                 AF.Identity)
                        nc.scalar.dma_start(out=vn_d[rt], in_=vb[:])

                    for s in (0, 4, 1, 5, 2, 6, 3, 7):
                        wsl = s2w.tile([128, 8, 256], F32R, tag="wqk")
                        nc.sync.dma_start(out=wsl[:], in_=wqk_in[s])
                        for sub in range(2):
                            m = 2 * s + sub
                            if m < 8:
                                for n in range(2):
                                    acc = s2p.tile([128, 512], F32, tag="qkacc")
                                    for ci in range(8):
                                        nc.tensor.matmul(
                                            acc[:], wsl[:, ci, sub * 128:(sub + 1) * 128],
                                            hT[:, ci, n * 512:(n + 1) * 512],
                                            start=(ci == 0), stop=(ci == 7))
                                    nc.scalar.activation(
                                        qT[:, m, n * 512:(n + 1) * 512], acc[:],
                                        AF.Identity, bias=bqk_sb[:, m:m + 1])
                            else:
                                ktb = s2ev.tile([128, T], F32R, tag="ktb")
                                for n in range(4):
                                    acc = s2p.tile([128, 512], F32, tag="qkacc")
                                    for ci in range(8):
                                        nc.tensor.matmul(
                                            acc[:], wsl[:, ci, sub * 128:(sub + 1) * 128],
                                            hT[:, ci, n * 512:(n + 1) * 512],
                                            start=(ci == 0), stop=(ci == 7))
                                    nc.scalar.activation(
                                        ktb[:, n * 512:(n + 1) * 512], acc[:],
                                        AF.Identity, bias=bqk_sb[:, m:m + 1])
                                nc.scalar.dma_start(out=kT_d[m - 8], in_=ktb[:])
            if debug:
                nc.gpsimd.dma_start(out=dbg["dbg_hT"][:], in_=hT[:])
                nc.gpsimd.dma_start(out=dbg["dbg_qT"][:], in_=qT[:])
            hT_pool.release()
            wv_pool.release()

            # ---- Stage 3: attention (scoresT layout, V augmented with ones col)
            mask_pool = tc.alloc_tile_pool(name="mask_pool", bufs=1, side="right")
            m_sb = mask_pool.tile([128, 2, 8, 512], F32R)
            nc.sync.dma_start(out=m_sb[:], in_=masks_in[:])
            yT_pool = tc.alloc_tile_pool(name="yT_pool", bufs=1)
            yT = yT_pool.tile([128, 8, R], F32R)

            with (
                tc.tile_pool(name="kv_pool", bufs=2) as kv_pool,
                tc.tile_pool(name="s3w", bufs=6) as s3w,
                tc.tile_pool(name="s3p", bufs=4, space="PSUM") as s3p,
                tc.tile_pool(name="s3yp", bufs=2, space="PSUM") as s3yp,
                tc.tile_pool(name="s3rp", bufs=2, space="PSUM") as s3rp,
            ):
                for hp in range(8):
                    kt = kv_pool.tile([128, T], F32R, tag="kt")
                    nc.sync.dma_start(out=kt[:], in_=kT_d[hp])
                    vn = kv_pool.tile([128, NCH, 2, HD + 1], F32R, tag="vn")
                    for sub_ in range(2):
                        off = hp * 128 + sub_ * HD
                        nc.sync.dma_start(
                            out=vn[:, :, sub_, 0:HD],
                            in_=vn_d[:, :, off:off + HD].rearrange("t p d -> p t d"))
                    with nc.allow_low_precision(reason="f32r ones fill"):
                        nc.scalar.activation(
                            vn[:, :, :, HD],
                            ones_f[:, :32].rearrange("p (t s) -> p t s", s=2), AF.Identity)
                    for sub in range(2):
                        ph = 64 * sub
                        for g in range(2):
                            qsl = qT[ph:ph + 64, hp, g * 512:(g + 1) * 512]
                            ya = s3yp.tile([65, 512], F32, tag="ya")
                            kcs = list(range(0, 4 * (g + 1))) + \
                                list(range(8, 8 + 4 * (g + 1)))
                            for idx, kc in enumerate(kcs):
                                if 4 * g <= kc < 4 * g + 4:
                                    mi = kc - 4 * g
                                elif 8 + 4 * g <= kc:
                                    mi = 4 + (kc - 8 - 4 * g)
                                else:
                                    mi = None
                                # causally-valid qi range (par=1 bound; masks
                                # zero the rest for par=0); keep >=256 for f32r
                                off = 0 if mi is None else min(mi % 4, 2) * 128
                                w = 512 - off
                                sc = s3p.tile([128, 512], F32, tag="sc")
                                nc.tensor.matmul(
                                    sc[:, :w], kt[ph:ph + 64, kc * 128:(kc + 1) * 128],
                                    qsl[:, off:], start=True, stop=True)
                                et = s3w.tile([128, 512], F32R, tag="et")
                                nc.scalar.activation(et[:, :w], sc[:, :w], AF.Exp)
                                if mi is not None:
                                    nc.vector.tensor_tensor(et[:, :w], et[:, :w],
                                                            m_sb[:, g, mi, off:],
                                                            ALU.mult)
                                nc.tensor.matmul(ya[:, off:], vn[:, kc, sub, :],
                                                 et[:, :w],
                                                 start=(idx == 0),
                                                 stop=(idx == len(kcs) - 1))
                            rec = s3w.tile([1, 512], F32R, tag="rec")
                            with nc.allow_low_precision(reason="f32r softmax recip"):
                                nc.vector.reciprocal(rec[:], ya[64:65, :])
                            rb = s3rp.tile([64, 512], F32, tag="rb")
                            nc.tensor.matmul(rb[:], ones_r[:, :64], rec[:],
                                             start=True, stop=True)
                            yf = s3w.tile([64, 512], F32, tag="yf")
                            nc.vector.tensor_copy(yf[:], ya[:64, :])
                            ytmp = s3w.tile([64, 512], F32R, tag="ytmp")
                            with nc.allow_low_precision(reason="f32r attn out"):
                                nc.vector.tensor_tensor(ytmp[:], yf[:], rb[:], ALU.mult)
                            nc.gpsimd.dma_start(
                                out=yT[ph:ph + 64, hp, g * 512:(g + 1) * 512],
                                in_=ytmp[:])

            if debug:
                nc.gpsimd.dma_start(out=dbg["dbg_yT"][:], in_=yT[:])
            mask_pool.release()
            qT_pool.release()

            # ---- Stage 4: proj + residual -> x1 (DRAM); LN2 -> h2T
            h2T_pool = tc.alloc_tile_pool(name="h2T_pool", bufs=1, side="right")
            h2T = h2T_pool.tile([128, 8, R], F32R)
            with (
                tc.tile_pool(name="s4wp", bufs=1) as s4wp,
                tc.tile_pool(name="s4w", bufs=2) as s4w,
                tc.tile_pool(name="s4pj", bufs=2) as s4pj,
                tc.tile_pool(name="s4s", bufs=3) as s4s,
                tc.tile_pool(name="s4p", bufs=3, space="PSUM") as s4p,
                tc.tile_pool(name="s4tp", bufs=2, space="PSUM") as s4tp,
            ):
                pjw = []
                for s in range(4):
                    w4 = s4wp.tile([128, 8, 256], F32R, tag=f"wpj{s}")
                    nc.sync.dma_start(out=w4[:], in_=wproj_in[s])
                    pjw.append(w4)
                for n in range(2):
                    pjt = []
                    for s in range(4):
                        for sub in range(2):
                            m = 2 * s + sub
                            acc = s4p.tile([128, 512], F32, tag="pjacc")
                            for ci in range(8):
                                nc.tensor.matmul(acc[:],
                                                 pjw[s][:, ci, sub * 128:(sub + 1) * 128],
                                                 yT[:, ci, n * 512:(n + 1) * 512],
                                                 start=(ci == 0), stop=(ci == 7))
                            ev = s4pj.tile([128, 512], F32, tag=f"pjev{m}")
                            nc.scalar.activation(ev[:], acc[:], AF.Identity,
                                                 bias=bproj_sb[:, m:m + 1])
                            pjt.append(ev)
                    for jj in range(4):
                        j = n * 4 + jj
                        xo = s4w.tile([128, C], F32, tag="xo")
                        nc.sync.dma_start(out=xo[:], in_=x_r[j])
                        x1t = s4w.tile([128, C], F32, tag="x1t")
                        for m in range(8):
                            pt = s4tp.tile([128, 128], F32, tag="pjt")
                            nc.tensor.transpose(
                                pt[:], pjt[m][:, jj * 128:(jj + 1) * 128], identity[:])
                            nc.vector.tensor_tensor(
                                x1t[:, m * 128:(m + 1) * 128],
                                pt[:], xo[:, m * 128:(m + 1) * 128], ALU.add)
                        nc.scalar.dma_start(out=x1_d[j], in_=x1t[:])
                        h2 = s4w.tile([128, C], F32, tag="h2")
                        _layernorm_tiles(nc, s4s, x1t[:], h2[:], "s4", eps_t[:])
                        for ci in range(8):
                            pt = s4tp.tile([128, 128], F32, tag="h2t")
                            nc.tensor.transpose(pt[:], h2[:, ci * 128:(ci + 1) * 128],
                                                identity[:])
                            nc.vector.tensor_copy(h2T[:, ci, j * 128:(j + 1) * 128], pt[:])

            if debug:
                nc.gpsimd.dma_start(out=dbg["dbg_h2T"][:], in_=h2T[:])
            yT_pool.release()

            # ---- Stage 5: MLP fc1 -> gelu -> gT (bf16)
            gT_pool = tc.alloc_tile_pool(name="gT_pool", bufs=1)
            gT = gT_pool.tile([128, 32, R], F32R)
            if True:
                with (
                    tc.tile_pool(name="s5w", bufs=3) as s5w,
                    tc.tile_pool(name="s5p", bufs=4, space="PSUM") as s5p,
                ):
                    for s in range(16):
                        wsl = s5w.tile([128, 8, 256], F32R, tag="wfc")
                        nc.gpsimd.dma_start(out=wsl[:], in_=wfc_in[s])
                        for sub in range(2):
                            m = 2 * s + sub
                            for n in range(2):
                                acc = s5p.tile([128, 512], F32, tag="facc")
                                for ci in range(8):
                                    nc.tensor.matmul(
                                        acc[:], wsl[:, ci, sub * 128:(sub + 1) * 128],
                                        h2T[:, ci, n * 512:(n + 1) * 512],
                                        start=(ci == 0), stop=(ci == 7))
                                nc.scalar.activation(
                                    gT[:, m, n * 512:(n + 1) * 512], acc[:],
                                    AF.Gelu_apprx_tanh, bias=bfc_sb[:, m:m + 1])
            if debug:
                nc.gpsimd.dma_start(out=dbg["dbg_gT"][:], in_=gT[:])
            h2T_pool.release()
            # ---- Stage 6: fc2 + residual -> out
            with (
                tc.tile_pool(name="s6w", bufs=2) as s6w,
                tc.tile_pool(name="s6ev", bufs=1) as s6ev,
                tc.tile_pool(name="s6p", bufs=2, space="PSUM") as s6p,
                tc.tile_pool(name="s6tp", bufs=2, space="PSUM") as s6tp,
            ):
                for half in range(2):
                    mev = []
                    for m2 in range(4):
                        m = half * 4 + m2
                        wsl = s6w.tile([128, 32, 128], F32R, tag="wfc2")
                        nc.scalar.dma_start(out=wsl[:], in_=wfc2_in[m])
                        acc = s6p.tile([128, 1024], F32, tag="macc")
                        for n in range(2):
                            for df in range(32):
                                nc.tensor.matmul(
                                    acc[:, n * 512:(n + 1) * 512],
                                    wsl[:, df, :],
                                    gT[:, df, n * 512:(n + 1) * 512],
                                    start=(df == 0), stop=(df == 31))
                        ev = s6ev.tile([128, 1024], F32, tag=f"mev{m2}")
                        nc.scalar.activation(ev[:], acc[:], AF.Identity,
                                             bias=bfc2_sb[:, m:m + 1])
                        mev.append(ev)
                    for j in range(8):
                        x1j = s6w.tile([128, 512], F32, tag="x1j")
                        nc.sync.dma_start(
                            out=x1j[:], in_=x1_d[j][:, half * 512:(half + 1) * 512])
                        stg = s6w.tile([128, 512], F32, tag="stg")
                        for m2 in range(4):
                            oc = half * 4 + m2
                            pt = s6tp.tile([128, 128], F32, tag="mt")
                            nc.tensor.transpose(
                                pt[:], mev[m2][:, j * 128:(j + 1) * 128], identity[:])
                            nc.vector.tensor_tensor(
                                stg[:, m2 * 128:(m2 + 1) * 128], pt[:],
                                x1j[:, m2 * 128:(m2 + 1) * 128], ALU.add)
                        nc.scalar.dma_start(
                            out=out_d[j * 128:(j + 1) * 128,
                                      half * 512:(half + 1) * 512],
                            in_=stg[:])
            gT_pool.release()

    nc.compile()
    return nc


_NC = None


def _host_prepare(x, ln1_w, ln1_b, w_attn, b_attn, w_proj, b_proj,
                  ln2_w, ln2_b, w_fc, b_fc, w_fc2, b_fc2):
    f32 = np.float32
    ln1_w = np.asarray(ln1_w, f32); ln1_b = np.asarray(ln1_b, f32)
    w_attn = np.asarray(w_attn, f32); b_attn = np.asarray(b_attn, f32)
    scale = np.zeros((3 * C,), f32)
    scale[:C] = 0.125
    scale[C:] = 1.0
    w_full = ln1_w[:, None] * w_attn * scale[None, :]
    b_full = (ln1_b @ w_attn + b_attn) * scale
    wv_pm = _f32r_round(w_full[:, 2 * C:]).reshape(8, 128, C).transpose(1, 0, 2)
    shared = {
        "wqk": _slab(_f32r_round(w_full[:, :2 * C]), 8, 256),
        "bqk": np.ascontiguousarray(b_full[:2 * C]),
        "wv": np.ascontiguousarray(wv_pm),
        "bv": _f32r_round(b_full[2 * C:]),
        "wproj": _slab(_f32r_round(np.asarray(w_proj, f32)), 8, 256),
        "bproj": np.asarray(b_proj, f32),
        "wfc": _slab(_f32r_round(np.asarray(ln2_w, f32)[:, None] * np.asarray(w_fc, f32)),
                     8, 256),
        "bfc": np.asarray(ln2_b, f32) @ np.asarray(w_fc, f32) + np.asarray(b_fc, f32),
        "wfc2": _slab(_f32r_round(np.asarray(w_fc2, f32)), 32, 128),
        "bfc2": np.asarray(b_fc2, f32),
    }
    # masks[p]: [128 (ki within chunk), 2 (qgroup), 8 (mask slot), 512 (qi)]
    masks = []
    ki = np.arange(128)
    for p in range(2):
        mk = np.zeros((128, 2, 8, 512), f32)
        for g in range(2):
            qc_i = np.arange(512) // 128          # i within group
            qr = np.arange(512) % 128
            q_real = (2 * (4 * g + qc_i) + p) * 128 + qr    # [512]
            for slot in range(8):
                if slot < 4:
                    real_chunk = 2 * (4 * g + slot) + p      # own keys
                else:
                    real_chunk = 2 * (4 * g + slot - 4) + 1 - p  # other-parity keys
                k_real = real_chunk * 128 + ki               # [128]
                mk[:, g, slot, :] = (k_real[:, None] <= q_real[None, :])
        masks.append(mk)
    return shared, masks


def kernel(**inputs):
    global _NC
    if _NC is None:
        _NC = build_program()
    nc = _NC
    x = np.asarray(inputs["x"], np.float32)
    shared, masks = _host_prepare(**inputs)
    in_maps = []
    for c in range(8):
        b, p = c // 2, c % 2
        perm = [2 * j + p for j in range(8)] + [2 * j + 1 - p for j in range(8)]
        xp = np.ascontiguousarray(
            x[b].reshape(NCH, 128, C)[perm].reshape(T, C))
        im = dict(shared)
        im["x"] = xp
        im["masks"] = masks[p]
        in_maps.append(im)
    res = run_bass_kernel_spmd(nc, in_maps, list(range(8)), trace=False).results
    out = np.empty((B, T, C), np.float32)
    for c in range(8):
        b, p = c // 2, c % 2
        oc = res[c]["out"].reshape(NOWN, 128, C)
        for j in range(NOWN):
            out[b, (2 * j + p) * 128:(2 * j + p + 1) * 128, :] = oc[j]
    return out



# revision 2
# speedup vs baseline: 1.0271x; 1.0271x over previous
"""TRN2 Bass kernel v2 for a GPT block (B=4, T=2048, C=1024, H=16, dff=4096).

Sharding: 8 cores, core c = (batch b=c//2, parity p=c%2); core owns the
interleaved 128-row chunks {2j+p} of batch b (own-prefix permuted x), computes
full-sequence k/v itself, produces its 1024 rows. One SPMD program; parity
differences enter only through data (the additive causal-fix mask).

Precision: fp8e4m3 DoubleRow matmuls (2 contraction subtiles per instr, 0.5
cycles/col) for ALL GEMMs. Attention runs pure fp8 (error ~2e-3). The MLP
uses split fp8 (hi + lo error-feedback on both activations and weights,
keeping hi*hi + lo*hi + hi*lo): error ~fp8^2, cost 3/4 of bf16.

Attention: scores are computed per head at PE quad positions (contraction
hd=64 packed as [32,2] DoubleRow pairs, 4 heads per 128-partition quad tile;
the q/k weight columns are host-permuted so GEMM outputs land directly in
quad layout). Key chunks are processed as pairs (own_s, perm-slot s+8) with
identical causal q-ranges for parity 1; a per-core additive mask fixes up
parity 0. Probabilities stay unnormalized (denominator = ones column in the
v operand); exp folds the score descale and fp8 range-scale into scale/bias.

Pipeline: LN1+v/k/q GEMMs -> attention for token-half 0 -> proj/LN2/fc1 of
half 0 interleaved with attention for half 1 -> fc2 half 0 -> MLP half 1.
"""
import numpy as np
import ml_dtypes

import concourse.bacc as bacc
import concourse.mybir as mybir
import concourse.tile as tile
from concourse.bass_utils import run_bass_kernel_spmd
from concourse.masks import make_identity

F32 = mybir.dt.float32
BF16 = mybir.dt.bfloat16
F8 = mybir.dt.float8e4
I8 = mybir.dt.int8
AF = mybir.ActivationFunctionType
ALU = mybir.AluOpType
DR = mybir.MatmulPerfMode.DoubleRow
NP8 = ml_dtypes.float8_e4m3

B, T, C, H, HD, DFF = 4, 2048, 1024, 16, 64, 4096
NCH = T // 128
NOWN = 8
R = NOWN * 128
EPS = 1e-5

S_H = 16.0       # h, h2 stored as value*16
S_W = 2048.0     # all weights stored as value*2048
S_Q = 64.0
S_K = 32.0
S_V = 16.0
LN_SE = float(np.log(4.0))   # et = exp(s)*4
SCH_A = 8.0 / float(np.log(2.0)) / (S_Q * S_K)   # schraudolph: bits = sc*A + B
SCH_B = 8.0 * (7.0 + 2.0)


def build_program(debug=False):
    nc = bacc.Bacc(None, target_bir_lowering=False, enable_partition_id=False)

    x_in = nc.declare_dram_parameter("x", [T, C], F32, isOutput=False)
    wq_in = nc.declare_dram_parameter("wq8", [8, 128, 4, 2, 128], F8, isOutput=False)
    wk_in = nc.declare_dram_parameter("wk8", [8, 128, 4, 2, 128], F8, isOutput=False)
    bqk_in = nc.declare_dram_parameter("bqk", [2, 8, 128], F32, isOutput=False)
    wv_in = nc.declare_dram_parameter("wv8", [128, 4, 2, C], F8, isOutput=False)
    bv_in = nc.declare_dram_parameter("bv8", [1, C], F8, isOutput=False)
    wp_in = nc.declare_dram_parameter("wp8", [8, 128, 4, 2, 128], F8, isOutput=False)
    bp_in = nc.declare_dram_parameter("bproj", [8, 128], F32, isOutput=False)
    wfh_in = nc.declare_dram_parameter("wf8hi", [32, 128, 4, 2, 128], F8, isOutput=False)
    wfl_in = nc.declare_dram_parameter("wf8lo", [32, 128, 4, 2, 128], F8, isOutput=False)
    bf_in = nc.declare_dram_parameter("bfc", [32, 128], F32, isOutput=False)
    w2h_in = nc.declare_dram_parameter("wf28hi", [8, 128, 16, 2, 128], F8, isOutput=False)
    w2l_in = nc.declare_dram_parameter("wf28lo", [8, 128, 16, 2, 128], F8, isOutput=False)
    b2_in = nc.declare_dram_parameter("bfc2", [8, 128], F32, isOutput=False)
    mk_in = nc.declare_dram_parameter("masks", [128, 2, 128], F8, isOutput=False)
    out_d = nc.declare_dram_parameter("out", [R, C], F32, isOutput=True)

    x_r = x_in[:].rearrange("(t p) c -> t p c", p=128)

    with tile.TileContext(nc) as tc:
        persist = tc.alloc_tile_pool(name="persist", bufs=1)
        ident = persist.tile([128, 128], BF16, tag="ident")
        make_identity(nc, ident[:])
        ones8 = persist.tile([1, 128], F8, tag="ones8")
        nc.gpsimd.memset(ones8[:], 1.0)

        def const(tag, val):
            t = persist.tile([128, 1], F32, tag=tag)
            nc.gpsimd.memset(t[:], float(val))
            return t
        c_eps = const("c_eps", EPS)
        c_exp = const("c_exp", 1.0 / (S_Q * S_K))
        c_ln4 = const("c_ln4", LN_SE)
        c_q = const("c_q", S_Q / (S_H * S_W))
        c_k = const("c_k", S_K / (S_H * S_W))
        c_v = const("c_v", S_V / (S_H * S_W))
        c_hw = const("c_hw", 1.0 / (S_H * S_W))
        c_gw = const("c_gw", 1.0 / S_W)

        bqk_sb = persist.tile([128, 2, 8], F32, tag="bqk")
        nc.sync.dma_start(out=bqk_sb[:], in_=bqk_in[:].rearrange("a s p -> p a s"))
        bp_sb = persist.tile([128, 8], F32, tag="bp")
        nc.sync.dma_start(out=bp_sb[:], in_=bp_in[:].rearrange("m p -> p m"))
        bf_sb = persist.tile([128, 32], F32, tag="bf")
        nc.sync.dma_start(out=bf_sb[:], in_=bf_in[:].rearrange("m p -> p m"))
        b2_sb = persist.tile([128, 8], F32, tag="b2")
        nc.sync.dma_start(out=b2_sb[:], in_=b2_in[:].rearrange("m p -> p m"))
        m12 = persist.tile([128, 2, 128], F8, tag="m12")
        nc.sync.dma_start(out=m12[:], in_=mk_in[:])
        bv8_sb = persist.tile([1, C], F8, tag="bv8")
        nc.sync.dma_start(out=bv8_sb[:], in_=bv_in[:])

        # right-stack pools: LIFO release order xo -> hT8 -> wqk -> kqv -> yT8
        yT_pool = tc.alloc_tile_pool(name="yT8", bufs=1, side="right")
        yT8 = yT_pool.tile([128, 8, R], F8, tag="yT8")
        kqv_pool = tc.alloc_tile_pool(name="kqv", bufs=1, side="right")
        kT = [kqv_pool.tile([128, 2, T], F8, tag=f"kT{qd}", name=f"kT{qd}")
              for qd in range(4)]
        qT = [kqv_pool.tile([128, 2, R], F8, tag=f"qT{qd}", name=f"qT{qd}")
              for qd in range(4)]
        vn = kqv_pool.tile([128, NCH, H, HD + 1], F8, tag="vn")
        with nc.allow_low_precision(reason="fp8 ones col"):
            nc.gpsimd.memset(vn[:, :, :, HD], 1.0)
        wqk_pool = tc.alloc_tile_pool(name="wqk", bufs=1, side="right")
        hT_pool = tc.alloc_tile_pool(name="hT8", bufs=1, side="right")
        hT8 = hT_pool.tile([128, 8, T], F8, tag="hT8")
        xo_pool = tc.alloc_tile_pool(name="xo", bufs=3, side="right")

        # input x first (startup critical path)
        xown = tc.alloc_tile_pool(name="xown", bufs=1)
        xts = []
        xo_tiles = []
        for rt in range(8):
            xa = xown.tile([128, C], F32, tag=f"x{rt}", name=f"x{rt}")
            nc.sync.dma_start(out=xa[:], in_=x_r[rt])
            xts.append(xa)
        for rt in range(8, 16):
            xb = xo_pool.tile([128, C], F32, tag=f"xt{rt % 3}", name=f"xt{rt}")
            nc.sync.dma_start(out=xb[:], in_=x_r[rt])
            xo_tiles.append(xb)

        wq_sb = wqk_pool.tile([128, 8, 4, 2, 128], F8, tag="wq")
        nc.scalar.dma_start(out=wq_sb[:], in_=wq_in[:].rearrange("s p c i m -> p s c i m"))
        wk_sb = wqk_pool.tile([128, 8, 4, 2, 128], F8, tag="wk")
        nc.scalar.dma_start(out=wk_sb[:], in_=wk_in[:].rearrange("s p c i m -> p s c i m"))
        wv_sb = wqk_pool.tile([128, 4, 2, C], F8, tag="wv")
        nc.scalar.dma_start(out=wv_sb[:], in_=wv_in[:])
        wp_pool = tc.alloc_tile_pool(name="wp", bufs=1)
        wp_sb = wp_pool.tile([128, 8, 4, 2, 128], F8, tag="wp")
        nc.scalar.dma_start(out=wp_sb[:], in_=wp_in[:].rearrange("s p c i m -> p s c i m"))


        # transient pools (A/B phases)
        s1s = tc.alloc_tile_pool(name="s1s", bufs=4)
        hb_pool = tc.alloc_tile_pool(name="hb", bufs=2)
        et_pool = tc.alloc_tile_pool(name="et", bufs=4)
        rec_pool = tc.alloc_tile_pool(name="rec", bufs=2)
        rbb_pool = tc.alloc_tile_pool(name="rbb", bufs=2)
        ytmp_pool = tc.alloc_tile_pool(name="ytmp", bufs=2)

        acc_pool = tc.alloc_tile_pool(name="acc", bufs=2, space="PSUM")
        sc_pool = tc.alloc_tile_pool(name="scp", bufs=3, space="PSUM")
        ya_pool = tc.alloc_tile_pool(name="yap", bufs=1, space="PSUM")
        pt_pool = tc.alloc_tile_pool(name="ptp", bufs=2, space="PSUM")

        # ---------------- helpers ----------------
        def layernorm_apply(x_ap, h_out, tag):
            """LN stats on DVE, rsqrt on ACT, apply via tensor_scalar -> bf16*S_H."""
            bns = s1s.tile([128, 2, 6], F32, tag=f"{tag}bns")
            nc.vector.bn_stats(bns[:, 0, :], x_ap[:, 0:512])
            nc.vector.bn_stats(bns[:, 1, :], x_ap[:, 512:1024])
            ag = s1s.tile([128, 2], F32, tag=f"{tag}ag")
            nc.vector.bn_aggr(ag[:], bns[:])
            sig = s1s.tile([128, 1], F32, tag=f"{tag}sg")
            nc.scalar.activation(sig[:], ag[:, 1:2], AF.Sqrt, bias=c_eps[:])
            rsig = s1s.tile([128, 1], F32, tag=f"{tag}rs")
            nc.vector.reciprocal(rsig[:], sig[:])
            s0 = s1s.tile([128, 1], F32, tag=f"{tag}s0")
            nc.vector.tensor_scalar(s0[:], rsig[:], S_H, None, ALU.mult)
            s1 = s1s.tile([128, 1], F32, tag=f"{tag}s1")
            nc.vector.scalar_tensor_tensor(s1[:], ag[:, 0:1], -S_H, rsig[:],
                                           op0=ALU.mult, op1=ALU.mult)
            nc.vector.tensor_scalar(h_out, x_ap, s0[:], s1[:], ALU.mult, ALU.add)

        def transpose_to(dst_ap_fn, src_bf, n_ci, evac):
            """Transpose [128, n_ci*128] bf16 -> PSUM, evac 4-ci groups."""
            for half in range((n_ci + 3) // 4):
                k = min(4, n_ci - half * 4)
                pt = pt_pool.tile([128, 4, 128], BF16, tag="pt")
                for i in range(k):
                    ci = half * 4 + i
                    nc.tensor.transpose(pt[:, i, :],
                                        src_bf[:, ci * 128:(ci + 1) * 128], ident[:])
                evac(pt, half, k)

        def emit_ln1_chunk(rt, x_ap):
            hb = hb_pool.tile([128, C], BF16, tag="hb")
            layernorm_apply(x_ap, hb[:], "s1")

            def evac(pt, half, k):
                with nc.allow_low_precision(reason="fp8 hT"):
                    nc.scalar.copy(
                        hT8[:, half * 4:half * 4 + k, rt * 128:(rt + 1) * 128],
                        pt[:, 0:k, :])
            transpose_to(None, hb[:], 8, evac)

        def emit_v_chunk(rt):
            for n in range(2):
                acc = acc_pool.tile([128, 512], F32, tag="acc")
                for c in range(4):
                    nc.tensor.matmul(acc[:],
                                     hT8[:, 2 * c:2 * c + 2, rt * 128:(rt + 1) * 128],
                                     wv_sb[:, c, :, n * 512:(n + 1) * 512],
                                     start=(c == 0), stop=False, perf_mode=DR)
                nc.tensor.matmul(acc[:], ones8[:],
                                 bv8_sb[:, n * 512:(n + 1) * 512],
                                 start=False, stop=True)
                with nc.allow_low_precision(reason="fp8 vn"):
                    nc.scalar.activation(
                        vn[:, rt, 8 * n:8 * n + 8, 0:HD], acc[:],
                        AF.Identity, scale=c_v[:])

        def emit_kq(which, qd, j, n):
            """k (which=0) or q (which=1) GEMM for slab (qd,j), 512-col block n."""
            w_sb = wk_sb if which == 0 else wq_sb
            dst = kT[qd] if which == 0 else qT[qd]
            acc = acc_pool.tile([128, 512], F32, tag="acc")
            for c in range(4):
                nc.tensor.matmul(acc[:],
                                 w_sb[:, qd * 2 + j, c, :, :],
                                 hT8[:, 2 * c:2 * c + 2, n * 512:(n + 1) * 512],
                                 start=(c == 0), stop=(c == 3), perf_mode=DR)
            with nc.allow_low_precision(reason="fp8 kq"):
                nc.scalar.activation(dst[:, j, n * 512:(n + 1) * 512], acc[:],
                                     AF.Identity,
                                     scale=(c_k if which == 0 else c_q)[:],
                                     bias=bqk_sb[:, which, qd * 2 + j:qd * 2 + j + 1])

        def emit_attn(h, g):
            """Attention for head h, q-block g (512 cols)."""
            qd, lane = h // 4, h % 4
            ph = lane * 32
            npairs = 4 * g + 4
            ya = ya_pool.tile([HD + 1, 512], F32, tag="ya")
            for s in range(npairs):
                off = max(0, s - 4 * g) * 128
                w = 512 - off
                et = et_pool.tile([128, 2, 512], F8, tag="et")
                for jj, slot in enumerate((s, 8 + s)):
                    sc = sc_pool.tile([128, 512], F32, tag="sc")
                    nc.tensor.matmul(
                        sc[:, 0:w],
                        kT[qd][ph:ph + 32, :, slot * 128:(slot + 1) * 128],
                        qT[qd][ph:ph + 32, :, g * 512 + off:(g + 1) * 512],
                        start=True, stop=True, perf_mode=DR,
                        tile_position=(ph, 0))
                    sel = (3 * h + 2 * s + jj) % 8
                    with nc.allow_low_precision(reason="fp8 exp"):
                        if sel < 4:
                            nc.scalar.activation(et[:, jj, 0:w], sc[:, 0:w],
                                                 AF.Exp, scale=c_exp[:],
                                                 bias=c_ln4[:])
                        else:
                            nc.vector.tensor_scalar(et[:, jj, 0:w].bitcast(I8),
                                                    sc[:, 0:w], SCH_A, SCH_B,
                                                    ALU.mult, ALU.add)
                if s >= 4 * g:
                    with nc.allow_low_precision(reason="fp8 mask"):
                        nc.gpsimd.tensor_tensor(et[:, :, 0:128], et[:, :, 0:128],
                                                m12[:], ALU.mult)
                nc.tensor.matmul(ya[:, off:512],
                                 vn[:, s:s + 9:8, h, :],
                                 et[:, :, 0:w],
                                 start=(s == 0), stop=(s == npairs - 1),
                                 perf_mode=DR)
            rec = rec_pool.tile([1, 512], BF16, tag="rec")
            rbb = rbb_pool.tile([HD, 512], BF16, tag="rbb")
            with nc.allow_low_precision(reason="bf16 softmax recip"):
                nc.vector.reciprocal(rec[:], ya[HD:HD + 1, :])
                nc.gpsimd.partition_broadcast(rbb[:], rec[:])
            if h % 2 == 0:
                with nc.allow_low_precision(reason="fp8 y"):
                    nc.vector.tensor_tensor(
                        yT8[0:HD, h // 2, g * 512:(g + 1) * 512],
                        ya[0:HD, :], rbb[:], ALU.mult)
            else:
                yt = ytmp_pool.tile([HD, 512], F8, tag="yt")
                with nc.allow_low_precision(reason="fp8 y"):
                    nc.vector.tensor_tensor(yt[:], ya[0:HD, :], rbb[:], ALU.mult)
                nc.gpsimd.dma_start(
                    out=yT8[HD:128, h // 2, g * 512:(g + 1) * 512], in_=yt[:])

        def emit_proj(m, half):
            acc = acc_pool.tile([128, 512], F32, tag="acc")
            for c in range(4):
                nc.tensor.matmul(acc[:],
                                 wp_sb[:, m, c, :, :],
                                 yT8[:, 2 * c:2 * c + 2, half * 512:(half + 1) * 512],
                                 start=(c == 0), stop=(c == 3), perf_mode=DR)
            pev = ev_pool.tile([128, 512], BF16, tag=f"ev{m}")
            nc.scalar.activation(pev[:], acc[:], AF.Identity, scale=c_hw[:],
                                 bias=bp_sb[:, m:m + 1])
            return pev

        def emit_x1_ln2(j, x_t, pevs, h2T8):
            """x1 = x + proj^T (in place into x_t), then LN2 -> h2T8 hi/lo."""
            jj = j % 4
            for half in range(2):
                pt = pt_pool.tile([128, 4, 128], BF16, tag="pt")
                for i in range(4):
                    m = half * 4 + i
                    nc.tensor.transpose(pt[:, i, :],
                                        pevs[m][:, jj * 128:(jj + 1) * 128], ident[:])
                nc.vector.tensor_tensor(x_t[:, half * 512:(half + 1) * 512],
                                        pt[:, :, :].rearrange("p a b -> p (a b)"),
                                        x_t[:, half * 512:(half + 1) * 512], ALU.add)
            hb = hb_pool.tile([128, C], BF16, tag="h2b")
            layernorm_apply(x_t[:], hb[:], "s4")

            def evac(pt, half, k):
                with nc.allow_low_precision(reason="fp8 h2"):
                    nc.scalar.copy(
                        h2T8[:, half * 4:half * 4 + 4, 0, jj * 128:(jj + 1) * 128],
                        pt[:, :, :])
                    nc.vector.tensor_tensor(
                        h2T8[:, half * 4:half * 4 + 4, 1, jj * 128:(jj + 1) * 128],
                        pt[:, :, :],
                        h2T8[:, half * 4:half * 4 + 4, 0, jj * 128:(jj + 1) * 128],
                        ALU.subtract)
            transpose_to(None, hb[:], 8, evac)

        def emit_fc1(m, h2T8, gT8, wfhi, wflo):
            acc = acc_pool.tile([128, 512], F32, tag="acc")
            for c in range(4):
                nc.tensor.matmul(acc[:], wfhi[:, c, :, :],
                                 h2T8[:, 2 * c:2 * c + 2, 0, :],
                                 start=(c == 0), stop=False, perf_mode=DR)
            for c in range(4):
                nc.tensor.matmul(acc[:], wfhi[:, c, :, :],
                                 h2T8[:, 2 * c:2 * c + 2, 1, :],
                                 start=False, stop=False, perf_mode=DR)
            for c in range(4):
                nc.tensor.matmul(acc[:], wflo[:, c, :, :],
                                 h2T8[:, 2 * c:2 * c + 2, 0, :],
                                 start=False, stop=(c == 3), perf_mode=DR)
            gf = gf_pool.tile([128, 512], BF16, tag="gf")
            nc.scalar.activation(gf[:], acc[:], AF.Gelu_apprx_tanh, scale=c_hw[:],
                                 bias=bf_sb[:, m:m + 1])
            with nc.allow_low_precision(reason="fp8 g"):
                nc.gpsimd.tensor_copy(gT8[:, m, 0, :], gf[:])
                nc.gpsimd.tensor_tensor(gT8[:, m, 1, :], gf[:], gT8[:, m, 0, :],
                                        ALU.subtract)

        def emit_fc2(m, gT8, w2hi, w2lo):
            acc = acc_pool.tile([128, 512], F32, tag="acc")
            for c in range(16):
                nc.tensor.matmul(acc[:], w2hi[:, c, :, :],
                                 gT8[:, 2 * c:2 * c + 2, 0, :],
                                 start=(c == 0), stop=False, perf_mode=DR)
            for c in range(16):
                nc.tensor.matmul(acc[:], w2hi[:, c, :, :],
                                 gT8[:, 2 * c:2 * c + 2, 1, :],
                                 start=False, stop=False, perf_mode=DR)
            for c in range(16):
                nc.tensor.matmul(acc[:], w2lo[:, c, :, :],
                                 gT8[:, 2 * c:2 * c + 2, 0, :],
                                 start=False, stop=(c == 15), perf_mode=DR)
            fev = ev_pool.tile([128, 512], BF16, tag=f"ev{m}")
            nc.scalar.activation(fev[:], acc[:], AF.Identity, scale=c_gw[:],
                                 bias=b2_sb[:, m:m + 1])
            return fev

        def emit_out(j, x_t, fevs):
            jj = j % 4
            stg = stg_pool.tile([128, C], F32, tag="stg")
            for half in range(2):
                pt = pt_pool.tile([128, 4, 128], BF16, tag="pt")
                for i in range(4):
                    m = half * 4 + i
                    nc.tensor.transpose(pt[:, i, :],
                                        fevs[m][:, jj * 128:(jj + 1) * 128], ident[:])
                nc.vector.tensor_tensor(stg[:, half * 512:(half + 1) * 512],
                                        pt[:, :, :].rearrange("p a b -> p (a b)"),
                                        x_t[:, half * 512:(half + 1) * 512], ALU.add)
            nc.sync.dma_start(out=out_d[j * 128:(j + 1) * 128, :], in_=stg[:])

        # ---------------- Phase A: LN1 + v/k/q ----------------
        for rt in range(16):
            xap = xts[rt][:] if rt < 8 else xo_tiles[rt - 8][:]
            emit_ln1_chunk(rt, xap)
            emit_v_chunk(rt)
        for n in (0, 2, 1, 3):
            for qd in range(4):
                for j in range(2):
                    emit_kq(0, qd, j, n)
        for n in range(2):
            for qd in range(4):
                for j in range(2):
                    emit_kq(1, qd, j, n)

        def load_wf(m):
            th = wf_pool.tile([128, 4, 2, 128], F8, tag="wfh")
            nc.sync.dma_start(out=th[:], in_=wfh_in[m].rearrange("p c i n -> p (c i n)")
                              .rearrange("p (c i n) -> p c i n", c=4, i=2))
            tl = wf_pool.tile([128, 4, 2, 128], F8, tag="wfl")
            nc.sync.dma_start(out=tl[:], in_=wfl_in[m].rearrange("p c i n -> p (c i n)")
                              .rearrange("p (c i n) -> p c i n", c=4, i=2))
            return th, tl

        def load_w2(m):
            th = w2_pool.tile([128, 16, 2, 128], F8, tag="w2h")
            nc.sync.dma_start(out=th[:], in_=w2h_in[m])
            tl = w2_pool.tile([128, 16, 2, 128], F8, tag="w2l")
            nc.sync.dma_start(out=tl[:], in_=w2l_in[m])
            return th, tl

        # ---------------- Phase B: attention g=0 ----------------
        xo_pool.release()
        for h in range(H):
            emit_attn(h, 0)
        hT_pool.release()
        wqk_pool.release()

        # ---------------- Phase C: half 0 MLP interleaved with attn g=1 ----
        h2_pool = tc.alloc_tile_pool(name="h2T8", bufs=1)
        g_pool = tc.alloc_tile_pool(name="gT8", bufs=1)
        ev_pool = tc.alloc_tile_pool(name="ev", bufs=1)
        gf_pool = tc.alloc_tile_pool(name="gf", bufs=3)
        stg_pool = tc.alloc_tile_pool(name="stg", bufs=2)
        wf_pool = tc.alloc_tile_pool(name="wf", bufs=4)
        w2_pool = tc.alloc_tile_pool(name="w2", bufs=2)
        h2T8_0 = h2_pool.tile([128, 8, 2, 512], F8, tag="h2T8")
        gT8_0 = g_pool.tile([128, 32, 2, 512], F8, tag="gT8")
        pevs = [emit_proj(m, 0) for m in range(8)]
        for j in range(4):
            emit_x1_ln2(j, xts[j][:], pevs, h2T8_0)

        wfs = [load_wf(0), load_wf(1)]
        for h in range(4):
            emit_attn(h, 1)
        for m in range(16):
            if m + 2 < 32:
                wfs.append(load_wf(m + 2))
            emit_fc1(m, h2T8_0, gT8_0, *wfs[m])
        for h in range(4, 8):
            emit_attn(h, 1)
        for m in range(16, 32):
            if m + 2 < 32:
                wfs.append(load_wf(m + 2))
            emit_fc1(m, h2T8_0, gT8_0, *wfs[m])
        for h in range(8, 12):
            emit_attn(h, 1)
        w2s = [load_w2(0), load_w2(1)]
        fevs = []
        for m in range(4):
            if m + 2 < 8:
                w2s.append(load_w2(m + 2))
            fevs.append(emit_fc2(m, gT8_0, *w2s[m]))
        for h in range(12, H):
            emit_attn(h, 1)
        for m in range(4, 8):
            if m + 2 < 8:
                w2s.append(load_w2(m + 2))
            fevs.append(emit_fc2(m, gT8_0, *w2s[m]))
        for j in range(4):
            emit_out(j, xts[j][:], fevs)
        kqv_pool.release()

        # ---------------- Phase D: half 1 MLP ----------------
        h2T8_1 = h2_pool.tile([128, 8, 2, 512], F8, tag="h2T8")
        gT8_1 = g_pool.tile([128, 32, 2, 512], F8, tag="gT8")
        pevs = [emit_proj(m, 1) for m in range(8)]
        yT_pool.release()
        for j in range(4, 8):
            emit_x1_ln2(j, xts[j][:], pevs, h2T8_1)
        wfs = [load_wf(0), load_wf(1)]
        for m in range(32):
            if m + 2 < 32:
                wfs.append(load_wf(m + 2))
            emit_fc1(m, h2T8_1, gT8_1, *wfs[m])
        w2s = [load_w2(0), load_w2(1)]
        fevs = []
        for m in range(8):
            if m + 2 < 8:
                w2s.append(load_w2(m + 2))
            fevs.append(emit_fc2(m, gT8_1, *w2s[m]))
        for j in range(4, 8):
            emit_out(j, xts[j][:], fevs)

        for pool in (w2_pool, wf_pool, stg_pool, gf_pool, ev_pool, g_pool,
                     h2_pool, ytmp_pool, rbb_pool, rec_pool, et_pool, hb_pool,
                     s1s, wp_pool, xown, persist,
                     pt_pool, ya_pool, sc_pool, acc_pool):
            pool.release()

    nc.compile()
    return nc


_NC = None


def _q8(x, scale):
    return (np.asarray(x, np.float32) * scale).astype(NP8)


def _host_prepare(x, ln1_w, ln1_b, w_attn, b_attn, w_proj, b_proj,
                  ln2_w, ln2_b, w_fc, b_fc, w_fc2, b_fc2):
    f32 = np.float32
    ln1_w = np.asarray(ln1_w, f32); ln1_b = np.asarray(ln1_b, f32)
    w_attn = np.asarray(w_attn, f32); b_attn = np.asarray(b_attn, f32)
    scl = np.ones((3 * C,), f32)
    scl[:C] = 0.125
    w_full = ln1_w[:, None] * w_attn * scl[None, :]
    b_full = (ln1_b @ w_attn + b_attn) * scl

    # quad column permutation: slab s=(qd,j), col = lane*32+r -> (4qd+lane)*64+j*32+r
    qcol = np.zeros((8, 128), np.int64)
    for qd in range(4):
        for j in range(2):
            for lane in range(4):
                for r in range(32):
                    qcol[qd * 2 + j, lane * 32 + r] = (4 * qd + lane) * 64 + j * 32 + r

    def slab_qk(w, b, s_out):
        # w [C, C] -> [8, 128, 4, 2, 128], b -> [8, 128]
        ws = np.zeros((8, 128, 4, 2, 128), f32)
        bs = np.zeros((8, 128), f32)
        for s in range(8):
            wsel = w[:, qcol[s]] * S_W          # [C, 128]
            ws[s] = wsel.reshape(4, 2, 128, 128).transpose(2, 0, 1, 3)
            bs[s] = b[qcol[s]] * s_out
        return _q8(ws, 1.0), bs * 1.0

    wq8, bq = slab_qk(w_full[:, :C], b_full[:C], S_Q)
    wk8, bk = slab_qk(w_full[:, C:2 * C], b_full[C:2 * C], S_K)
    bqk = np.stack([bk, bq])    # [2, 8, 128]: [0]=k, [1]=q

    wv8 = _q8(w_full[:, 2 * C:].reshape(4, 2, 128, C).transpose(2, 0, 1, 3), S_W)
    bv8 = _q8(b_full[2 * C:].reshape(1, C), S_H * S_W)

    wp = np.asarray(w_proj, f32)
    wp8 = _q8(wp.reshape(4, 2, 128, 8, 128).transpose(3, 2, 0, 1, 4), S_W)
    bproj = np.asarray(b_proj, f32).reshape(8, 128)

    wf_eff = np.asarray(ln2_w, f32)[:, None] * np.asarray(w_fc, f32)
    bfc = (np.asarray(ln2_b, f32) @ np.asarray(w_fc, f32) +
           np.asarray(b_fc, f32)).reshape(32, 128)
    wfs = (wf_eff * S_W).reshape(4, 2, 128, 32, 128).transpose(3, 2, 0, 1, 4)
    wf8hi = wfs.astype(NP8)
    wf8lo = (wfs - wf8hi.astype(f32)).astype(NP8)

    w2 = np.asarray(w_fc2, f32)
    w2s = (w2 * S_W).reshape(16, 2, 128, 8, 128).transpose(3, 2, 0, 1, 4)
    wf28hi = w2s.astype(NP8)
    wf28lo = (w2s - wf28hi.astype(f32)).astype(NP8)
    bfc2 = np.asarray(b_fc2, f32).reshape(8, 128)

    shared = {
        "wq8": wq8, "wk8": wk8, "bqk": bqk, "wv8": wv8, "bv8": bv8,
        "wp8": wp8, "bproj": bproj,
        "wf8hi": wf8hi, "wf8lo": wf8lo, "bfc": bfc,
        "wf28hi": wf28hi, "wf28lo": wf28lo, "bfc2": bfc2,
    }
    # masks[p]: [128, 2, 128] fp8 multiplicative: [.,0,.]=lower-tri, [.,1,.]=parity
    ki = np.arange(128)[:, None]
    qi = np.arange(128)[None, :]
    diag = (ki <= qi).astype(f32)
    masks = []
    for p in range(2):
        m2 = np.full((128, 128), 1.0 if p == 1 else 0.0, f32)
        masks.append(np.stack([diag, m2], axis=1).astype(NP8))
    return shared, masks


def kernel(**inputs):
    global _NC
    if _NC is None:
        _NC = build_program()
    nc = _NC
    x = np.asarray(inputs["x"], np.float32)
    shared, masks = _host_prepare(**inputs)
    in_maps = []
    for c in range(8):
        b, p = c // 2, c % 2
        perm = [2 * j + p for j in range(8)] + [2 * j + 1 - p for j in range(8)]
        xp = np.ascontiguousarray(x[b].reshape(NCH, 128, C)[perm].reshape(T, C))
        im = dict(shared)
        im["x"] = xp
        im["masks"] = masks[p]
        in_maps.append(im)
    res = run_bass_kernel_spmd(nc, in_maps, list(range(8)), trace=False).results
    out = np.empty((B, T, C), np.float32)
    for c in range(8):
        b, p = c // 2, c % 2
        oc = res[c]["out"].reshape(NOWN, 128, C)
        for j in range(NOWN):
            out[b, (2 * j + p) * 128:(2 * j + p + 1) * 128, :] = oc[j]
    return out


# revision 4
# speedup vs baseline: 1.0969x; 1.0681x over previous
"""TRN2 Bass kernel v2 for a GPT block (B=4, T=2048, C=1024, H=16, dff=4096).

Sharding: 8 cores, core c = (batch b=c//2, parity p=c%2); core owns the
interleaved 128-row chunks {2j+p} of batch b (own-prefix permuted x), computes
full-sequence k/v itself, produces its 1024 rows. One SPMD program; parity
differences enter only through data (the additive causal-fix mask).

Precision: fp8e4m3 DoubleRow matmuls (2 contraction subtiles per instr, 0.5
cycles/col) for ALL GEMMs. Attention runs pure fp8 (error ~2e-3). The MLP
uses split fp8 (hi + lo error-feedback on both activations and weights,
keeping hi*hi + lo*hi + hi*lo): error ~fp8^2, cost 3/4 of bf16.

Attention: scores are computed per head at PE quad positions (contraction
hd=64 packed as [32,2] DoubleRow pairs, 4 heads per 128-partition quad tile;
the q/k weight columns are host-permuted so GEMM outputs land directly in
quad layout). Key chunks are processed as pairs (own_s, perm-slot s+8) with
identical causal q-ranges for parity 1; a per-core additive mask fixes up
parity 0. Probabilities stay unnormalized (denominator = ones column in the
v operand); exp folds the score descale and fp8 range-scale into scale/bias.

Pipeline: LN1+v/k/q GEMMs -> attention for token-half 0 -> proj/LN2/fc1 of
half 0 interleaved with attention for half 1 -> fc2 half 0 -> MLP half 1.
"""
import numpy as np
import ml_dtypes

import concourse.bacc as bacc
import concourse.mybir as mybir
import concourse.tile as tile
from concourse.bass_utils import run_bass_kernel_spmd
from concourse.masks import make_identity

F32 = mybir.dt.float32
BF16 = mybir.dt.bfloat16
F8 = mybir.dt.float8e4
I8 = mybir.dt.int8
AF = mybir.ActivationFunctionType
ALU = mybir.AluOpType
DR = mybir.MatmulPerfMode.DoubleRow
NP8 = ml_dtypes.float8_e4m3

B, T, C, H, HD, DFF = 4, 2048, 1024, 16, 64, 4096
NCH = T // 128
NOWN = 8
R = NOWN * 128
EPS = 1e-5

S_H = 16.0       # h, h2 stored as value*16
S_W = 2048.0     # all weights stored as value*2048
S_Q = 64.0
S_K = 32.0
S_V = 16.0
LN_SE = float(np.log(4.0))   # et = exp(s)*4
SCH_A = 8.0 / float(np.log(2.0)) / (S_Q * S_K)   # schraudolph: bits = sc*A + B
SCH_B = 8.0 * (7.0 + 2.0)


def build_program(debug=False):
    nc = bacc.Bacc(None, target_bir_lowering=False, enable_partition_id=False)

    x_in = nc.declare_dram_parameter("x", [T, C], F32, isOutput=False)
    wq_in = nc.declare_dram_parameter("wq8", [8, 128, 4, 2, 128], F8, isOutput=False)
    wk_in = nc.declare_dram_parameter("wk8", [8, 128, 4, 2, 128], F8, isOutput=False)
    bqk_in = nc.declare_dram_parameter("bqk", [2, 8, 128], F32, isOutput=False)
    wv_in = nc.declare_dram_parameter("wv8", [128, 4, 2, C], F8, isOutput=False)
    bv_in = nc.declare_dram_parameter("bv8", [1, C], F8, isOutput=False)
    wp_in = nc.declare_dram_parameter("wp8", [8, 128, 4, 2, 128], F8, isOutput=False)
    bp_in = nc.declare_dram_parameter("bproj", [8, 128], F32, isOutput=False)
    wfh_in = nc.declare_dram_parameter("wf8hi", [32, 128, 4, 2, 128], F8, isOutput=False)
    wfl_in = nc.declare_dram_parameter("wf8lo", [32, 128, 4, 2, 128], F8, isOutput=False)
    bf_in = nc.declare_dram_parameter("bfc", [32, 128], F32, isOutput=False)
    w2h_in = nc.declare_dram_parameter("wf28hi", [8, 128, 16, 2, 128], F8, isOutput=False)
    w2l_in = nc.declare_dram_parameter("wf28lo", [8, 128, 16, 2, 128], F8, isOutput=False)
    b2_in = nc.declare_dram_parameter("bfc2", [8, 128], F32, isOutput=False)
    mk_in = nc.declare_dram_parameter("masks", [128, 2, 128], F8, isOutput=False)
    out_d = nc.declare_dram_parameter("out", [R, C], F32, isOutput=True)

    x_r = x_in[:].rearrange("(t p) c -> t p c", p=128)

    with tile.TileContext(nc) as tc:
        persist = tc.alloc_tile_pool(name="persist", bufs=1)
        ident = persist.tile([128, 128], BF16, tag="ident")
        make_identity(nc, ident[:])
        ones8 = persist.tile([1, 128], F8, tag="ones8")
        nc.gpsimd.memset(ones8[:], 1.0)

        def const(tag, val):
            t = persist.tile([128, 1], F32, tag=tag)
            nc.gpsimd.memset(t[:], float(val))
            return t
        c_eps = const("c_eps", EPS)
        c_exp = const("c_exp", 1.0 / (S_Q * S_K))
        c_ln4 = const("c_ln4", LN_SE)
        c_q = const("c_q", S_Q / (S_H * S_W))
        c_k = const("c_k", S_K / (S_H * S_W))
        c_v = const("c_v", S_V / (S_H * S_W))
        c_hw = const("c_hw", 1.0 / (S_H * S_W))
        c_gw = const("c_gw", 1.0 / S_W)

        bqk_sb = persist.tile([128, 2, 8], F32, tag="bqk")
        nc.sync.dma_start(out=bqk_sb[:], in_=bqk_in[:].rearrange("a s p -> p a s"))
        bp_sb = persist.tile([128, 8], F32, tag="bp")
        nc.sync.dma_start(out=bp_sb[:], in_=bp_in[:].rearrange("m p -> p m"))
        bf_sb = persist.tile([128, 32], F32, tag="bf")
        nc.sync.dma_start(out=bf_sb[:], in_=bf_in[:].rearrange("m p -> p m"))
        b2_sb = persist.tile([128, 8], F32, tag="b2")
        nc.sync.dma_start(out=b2_sb[:], in_=b2_in[:].rearrange("m p -> p m"))
        m12 = persist.tile([128, 2, 128], F8, tag="m12")
        nc.sync.dma_start(out=m12[:], in_=mk_in[:])
        bv8_sb = persist.tile([1, C], F8, tag="bv8")
        nc.sync.dma_start(out=bv8_sb[:], in_=bv_in[:])

        # right-stack pools: LIFO release order xo -> hT8 -> wqk -> kqv -> yT8
        yT_pool = tc.alloc_tile_pool(name="yT8", bufs=1, side="right")
        yT8 = yT_pool.tile([128, 8, R], F8, tag="yT8")
        kqv_pool = tc.alloc_tile_pool(name="kqv", bufs=1, side="right")
        kT = [kqv_pool.tile([128, 2, T], F8, tag=f"kT{qd}", name=f"kT{qd}")
              for qd in range(4)]
        qT = [kqv_pool.tile([128, 2, R], F8, tag=f"qT{qd}", name=f"qT{qd}")
              for qd in range(4)]
        vn = kqv_pool.tile([128, NCH, H, HD + 1], F8, tag="vn")
        with nc.allow_low_precision(reason="fp8 ones col"):
            nc.gpsimd.memset(vn[:, :, :, HD], 1.0)
        wqk_pool = tc.alloc_tile_pool(name="wqk", bufs=1, side="right")
        hT_pool = tc.alloc_tile_pool(name="hT8", bufs=1, side="right")
        hT8 = hT_pool.tile([128, 8, T], F8, tag="hT8")
        xo_pool = tc.alloc_tile_pool(name="xo", bufs=3, side="right")

        # input x first (startup critical path)
        xown = tc.alloc_tile_pool(name="xown", bufs=1)
        xts = []
        xo_tiles = []
        for rt in range(8):
            xa = xown.tile([128, C], F32, tag=f"x{rt}", name=f"x{rt}")
            nc.sync.dma_start(out=xa[:], in_=x_r[rt])
            xts.append(xa)
        for rt in range(8, 16):
            xb = xo_pool.tile([128, C], F32, tag=f"xt{rt % 3}", name=f"xt{rt}")
            nc.sync.dma_start(out=xb[:], in_=x_r[rt])
            xo_tiles.append(xb)

        wq_sb = wqk_pool.tile([128, 8, 4, 2, 128], F8, tag="wq")
        nc.scalar.dma_start(out=wq_sb[:], in_=wq_in[:].rearrange("s p c i m -> p s c i m"))
        wk_sb = wqk_pool.tile([128, 8, 4, 2, 128], F8, tag="wk")
        nc.scalar.dma_start(out=wk_sb[:], in_=wk_in[:].rearrange("s p c i m -> p s c i m"))
        wv_sb = wqk_pool.tile([128, 4, 2, C], F8, tag="wv")
        nc.scalar.dma_start(out=wv_sb[:], in_=wv_in[:])
        wp_pool = tc.alloc_tile_pool(name="wp", bufs=1)
        wp_sb = wp_pool.tile([128, 8, 4, 2, 128], F8, tag="wp")
        nc.scalar.dma_start(out=wp_sb[:], in_=wp_in[:].rearrange("s p c i m -> p s c i m"))


        # transient pools (A/B phases)
        s1s = tc.alloc_tile_pool(name="s1s", bufs=4)
        hb_pool = tc.alloc_tile_pool(name="hb", bufs=2)
        et_pool = tc.alloc_tile_pool(name="et", bufs=4)
        rec_pool = tc.alloc_tile_pool(name="rec", bufs=2)
        rbb_pool = tc.alloc_tile_pool(name="rbb", bufs=2)
        ytmp_pool = tc.alloc_tile_pool(name="ytmp", bufs=2)

        acc_pool = tc.alloc_tile_pool(name="acc", bufs=2, space="PSUM")
        sc_pool = tc.alloc_tile_pool(name="scp", bufs=3, space="PSUM")
        ya_pool = tc.alloc_tile_pool(name="yap", bufs=1, space="PSUM")
        pt_pool = tc.alloc_tile_pool(name="ptp", bufs=2, space="PSUM")

        # ---------------- helpers ----------------
        def layernorm_apply(x_ap, h_out, tag):
            """LN stats on DVE, rsqrt on ACT, apply via tensor_scalar -> bf16*S_H."""
            bns = s1s.tile([128, 2, 6], F32, tag=f"{tag}bns")
            nc.vector.bn_stats(bns[:, 0, :], x_ap[:, 0:512])
            nc.vector.bn_stats(bns[:, 1, :], x_ap[:, 512:1024])
            ag = s1s.tile([128, 2], F32, tag=f"{tag}ag")
            nc.vector.bn_aggr(ag[:], bns[:])
            sig = s1s.tile([128, 1], F32, tag=f"{tag}sg")
            nc.scalar.activation(sig[:], ag[:, 1:2], AF.Sqrt, bias=c_eps[:])
            rsig = s1s.tile([128, 1], F32, tag=f"{tag}rs")
            nc.vector.reciprocal(rsig[:], sig[:])
            s0 = s1s.tile([128, 1], F32, tag=f"{tag}s0")
            nc.vector.tensor_scalar(s0[:], rsig[:], S_H, None, ALU.mult)
            s1 = s1s.tile([128, 1], F32, tag=f"{tag}s1")
            nc.vector.scalar_tensor_tensor(s1[:], ag[:, 0:1], -S_H, rsig[:],
                                           op0=ALU.mult, op1=ALU.mult)
            nc.vector.tensor_scalar(h_out, x_ap, s0[:], s1[:], ALU.mult, ALU.add)

        def transpose_to(dst_ap_fn, src_bf, n_ci, evac):
            """Transpose [128, n_ci*128] bf16 -> PSUM, evac 4-ci groups."""
            for half in range((n_ci + 3) // 4):
                k = min(4, n_ci - half * 4)
                pt = pt_pool.tile([128, 4, 128], BF16, tag="pt")
                for i in range(k):
                    ci = half * 4 + i
                    nc.tensor.transpose(pt[:, i, :],
                                        src_bf[:, ci * 128:(ci + 1) * 128], ident[:])
                evac(pt, half, k)

        def emit_ln1_chunk(rt, x_ap):
            hb = hb_pool.tile([128, C], BF16, tag="hb")
            layernorm_apply(x_ap, hb[:], "s1")

            def evac(pt, half, k):
                with nc.allow_low_precision(reason="fp8 hT"):
                    nc.scalar.copy(
                        hT8[:, half * 4:half * 4 + k, rt * 128:(rt + 1) * 128],
                        pt[:, 0:k, :])
            transpose_to(None, hb[:], 8, evac)

        def emit_v_chunk(rt):
            for n in range(2):
                acc = acc_pool.tile([128, 512], F32, tag="acc")
                for c in range(4):
                    nc.tensor.matmul(acc[:],
                                     hT8[:, 2 * c:2 * c + 2, rt * 128:(rt + 1) * 128],
                                     wv_sb[:, c, :, n * 512:(n + 1) * 512],
                                     start=(c == 0), stop=False, perf_mode=DR)
                nc.tensor.matmul(acc[:], ones8[:],
                                 bv8_sb[:, n * 512:(n + 1) * 512],
                                 start=False, stop=True)
                with nc.allow_low_precision(reason="fp8 vn"):
                    nc.scalar.activation(
                        vn[:, rt, 8 * n:8 * n + 8, 0:HD], acc[:],
                        AF.Identity, scale=c_v[:])

        def emit_kq(which, qd, j, n):
            """k (which=0) or q (which=1) GEMM for slab (qd,j), 512-col block n."""
            w_sb = wk_sb if which == 0 else wq_sb
            dst = kT[qd] if which == 0 else qT[qd]
            acc = acc_pool.tile([128, 512], F32, tag="acc")
            for c in range(4):
                nc.tensor.matmul(acc[:],
                                 w_sb[:, qd * 2 + j, c, :, :],
                                 hT8[:, 2 * c:2 * c + 2, n * 512:(n + 1) * 512],
                                 start=(c == 0), stop=(c == 3), perf_mode=DR)
            with nc.allow_low_precision(reason="fp8 kq"):
                nc.scalar.activation(dst[:, j, n * 512:(n + 1) * 512], acc[:],
                                     AF.Identity,
                                     scale=(c_k if which == 0 else c_q)[:],
                                     bias=bqk_sb[:, which, qd * 2 + j:qd * 2 + j + 1])

        def emit_attn(h, g):
            """Attention for head h, q-block g (512 cols)."""
            qd, lane = h // 4, h % 4
            ph = lane * 32
            npairs = 4 * g + 4
            ya = ya_pool.tile([HD + 1, 512], F32, tag="ya")
            for s in range(npairs):
                off = max(0, s - 4 * g) * 128
                w = 512 - off
                et = et_pool.tile([128, 2, 512], F8, tag="et")
                for jj, slot in enumerate((s, 8 + s)):
                    sc = sc_pool.tile([128, 512], F32, tag="sc")
                    nc.tensor.matmul(
                        sc[:, 0:w],
                        kT[qd][ph:ph + 32, :, slot * 128:(slot + 1) * 128],
                        qT[qd][ph:ph + 32, :, g * 512 + off:(g + 1) * 512],
                        start=True, stop=True, perf_mode=DR,
                        tile_position=(ph, 0))
                    sel = (3 * h + 2 * s + jj) % 8
                    with nc.allow_low_precision(reason="fp8 exp"):
                        if sel < 4:
                            nc.scalar.activation(et[:, jj, 0:w], sc[:, 0:w],
                                                 AF.Exp, scale=c_exp[:],
                                                 bias=c_ln4[:])
                        else:
                            nc.vector.tensor_scalar(et[:, jj, 0:w].bitcast(I8),
                                                    sc[:, 0:w], SCH_A, SCH_B,
                                                    ALU.mult, ALU.add)
                if s >= 4 * g:
                    with nc.allow_low_precision(reason="fp8 mask"):
                        nc.gpsimd.tensor_tensor(et[:, :, 0:128], et[:, :, 0:128],
                                                m12[:], ALU.mult)
                nc.tensor.matmul(ya[:, off:512],
                                 vn[:, s:s + 9:8, h, :],
                                 et[:, :, 0:w],
                                 start=(s == 0), stop=(s == npairs - 1),
                                 perf_mode=DR)
            rec = rec_pool.tile([1, 512], BF16, tag="rec")
            rbb = rbb_pool.tile([HD, 512], BF16, tag="rbb")
            with nc.allow_low_precision(reason="bf16 softmax recip"):
                nc.vector.reciprocal(rec[:], ya[HD:HD + 1, :])
                nc.gpsimd.partition_broadcast(rbb[:], rec[:])
            if h % 2 == 0:
                with nc.allow_low_precision(reason="fp8 y"):
                    nc.vector.tensor_tensor(
                        yT8[0:HD, h // 2, g * 512:(g + 1) * 512],
                        ya[0:HD, :], rbb[:], ALU.mult)
            else:
                yt = ytmp_pool.tile([HD, 512], F8, tag="yt")
                with nc.allow_low_precision(reason="fp8 y"):
                    nc.vector.tensor_tensor(yt[:], ya[0:HD, :], rbb[:], ALU.mult)
                nc.gpsimd.dma_start(
                    out=yT8[HD:128, h // 2, g * 512:(g + 1) * 512], in_=yt[:])

        def emit_proj(m, half):
            acc = acc_pool.tile([128, 512], F32, tag="acc")
            for c in range(4):
                nc.tensor.matmul(acc[:],
                                 wp_sb[:, m, c, :, :],
                                 yT8[:, 2 * c:2 * c + 2, half * 512:(half + 1) * 512],
                                 start=(c == 0), stop=(c == 3), perf_mode=DR)
            pev = ev_pool.tile([128, 512], BF16, tag=f"ev{m}")
            nc.scalar.activation(pev[:], acc[:], AF.Identity, scale=c_hw[:],
                                 bias=bp_sb[:, m:m + 1])
            return pev

        def emit_x1_ln2(j, x_t, pevs, h2T8):
            """x1 = x + proj^T (in place into x_t), then LN2 -> h2T8 hi/lo."""
            jj = j % 4
            for half in range(2):
                pt = pt_pool.tile([128, 4, 128], BF16, tag="pt")
                for i in range(4):
                    m = half * 4 + i
                    nc.tensor.transpose(pt[:, i, :],
                                        pevs[m][:, jj * 128:(jj + 1) * 128], ident[:])
                nc.vector.tensor_tensor(x_t[:, half * 512:(half + 1) * 512],
                                        pt[:, :, :].rearrange("p a b -> p (a b)"),
                                        x_t[:, half * 512:(half + 1) * 512], ALU.add)
            hb = hb_pool.tile([128, C], BF16, tag="h2b")
            layernorm_apply(x_t[:], hb[:], "s4")

            def evac(pt, half, k):
                with nc.allow_low_precision(reason="fp8 h2"):
                    nc.scalar.copy(
                        h2T8[:, half * 4:half * 4 + 4, 0, jj * 128:(jj + 1) * 128],
                        pt[:, :, :])
                    nc.vector.tensor_tensor(
                        h2T8[:, half * 4:half * 4 + 4, 1, jj * 128:(jj + 1) * 128],
                        pt[:, :, :],
                        h2T8[:, half * 4:half * 4 + 4, 0, jj * 128:(jj + 1) * 128],
                        ALU.subtract)
            transpose_to(None, hb[:], 8, evac)

        def emit_fc1(m, h2T8, gT8, wfhi, wflo):
            acc = acc_pool.tile([128, 512], F32, tag="acc")
            for c in range(4):
                nc.tensor.matmul(acc[:], wfhi[:, c, :, :],
                                 h2T8[:, 2 * c:2 * c + 2, 0, :],
                                 start=(c == 0), stop=False, perf_mode=DR)
            for c in range(4):
                nc.tensor.matmul(acc[:], wfhi[:, c, :, :],
                                 h2T8[:, 2 * c:2 * c + 2, 1, :],
                                 start=False, stop=False, perf_mode=DR)
            for c in range(4):
                nc.tensor.matmul(acc[:], wflo[:, c, :, :],
                                 h2T8[:, 2 * c:2 * c + 2, 0, :],
                                 start=False, stop=(c == 3), perf_mode=DR)
            with nc.allow_low_precision(reason="fp8 g"):
                nc.scalar.activation(gT8[:, m, 0, :], acc[:], AF.Gelu_apprx_tanh,
                                     scale=c_hw[:], bias=bf_sb[:, m:m + 1])

        def emit_fc2(m, gT8, w2hi, w2lo):
            acc = acc_pool.tile([128, 512], F32, tag="acc")
            for c in range(16):
                nc.tensor.matmul(acc[:], w2hi[:, c, :, :],
                                 gT8[:, 2 * c:2 * c + 2, 0, :],
                                 start=(c == 0), stop=False, perf_mode=DR)
            for c in range(16):
                nc.tensor.matmul(acc[:], w2lo[:, c, :, :],
                                 gT8[:, 2 * c:2 * c + 2, 0, :],
                                 start=False, stop=(c == 15), perf_mode=DR)
            fev = ev_pool.tile([128, 512], BF16, tag=f"ev{m}")
            nc.scalar.activation(fev[:], acc[:], AF.Identity, scale=c_gw[:],
                                 bias=b2_sb[:, m:m + 1])
            return fev

        def emit_out(j, x_t, fevs):
            jj = j % 4
            stg = stg_pool.tile([128, C], F32, tag="stg")
            for half in range(2):
                pt = pt_pool.tile([128, 4, 128], BF16, tag="pt")
                for i in range(4):
                    m = half * 4 + i
                    nc.tensor.transpose(pt[:, i, :],
                                        fevs[m][:, jj * 128:(jj + 1) * 128], ident[:])
                nc.vector.tensor_tensor(stg[:, half * 512:(half + 1) * 512],
                                        pt[:, :, :].rearrange("p a b -> p (a b)"),
                                        x_t[:, half * 512:(half + 1) * 512], ALU.add)
            nc.sync.dma_start(out=out_d[j * 128:(j + 1) * 128, :], in_=stg[:])

        # ---------------- Phase A: LN1 + v/k/q ----------------
        for rt in range(16):
            xap = xts[rt][:] if rt < 8 else xo_tiles[rt - 8][:]
            emit_ln1_chunk(rt, xap)
            emit_v_chunk(rt)
        for n in (0, 2, 1, 3):
            for qd in range(4):
                for j in range(2):
                    emit_kq(0, qd, j, n)
        for n in range(2):
            for qd in range(4):
                for j in range(2):
                    emit_kq(1, qd, j, n)

        def load_wf(m):
            th = wf_pool.tile([128, 4, 2, 128], F8, tag="wfh")
            nc.sync.dma_start(out=th[:], in_=wfh_in[m].rearrange("p c i n -> p (c i n)")
                              .rearrange("p (c i n) -> p c i n", c=4, i=2))
            tl = wf_pool.tile([128, 4, 2, 128], F8, tag="wfl")
            nc.sync.dma_start(out=tl[:], in_=wfl_in[m].rearrange("p c i n -> p (c i n)")
                              .rearrange("p (c i n) -> p c i n", c=4, i=2))
            return th, tl

        def load_w2(m):
            th = w2_pool.tile([128, 16, 2, 128], F8, tag="w2h")
            nc.sync.dma_start(out=th[:], in_=w2h_in[m])
            tl = w2_pool.tile([128, 16, 2, 128], F8, tag="w2l")
            nc.sync.dma_start(out=tl[:], in_=w2l_in[m])
            return th, tl

        # ---------------- Phase B: attention g=0 ----------------
        xo_pool.release()
        for h in range(H):
            emit_attn(h, 0)
        hT_pool.release()
        wqk_pool.release()

        # ---------------- Phase C: half 0 MLP interleaved with attn g=1 ----
        h2_pool = tc.alloc_tile_pool(name="h2T8", bufs=1)
        g_pool = tc.alloc_tile_pool(name="gT8", bufs=1)
        ev_pool = tc.alloc_tile_pool(name="ev", bufs=1)
        gf_pool = tc.alloc_tile_pool(name="gf", bufs=3)
        stg_pool = tc.alloc_tile_pool(name="stg", bufs=2)
        wf_pool = tc.alloc_tile_pool(name="wf", bufs=4)
        w2_pool = tc.alloc_tile_pool(name="w2", bufs=2)
        h2T8_0 = h2_pool.tile([128, 8, 2, 512], F8, tag="h2T8")
        gT8_0 = g_pool.tile([128, 32, 2, 512], F8, tag="gT8")
        pevs = [emit_proj(m, 0) for m in range(8)]
        for j in range(4):
            emit_x1_ln2(j, xts[j][:], pevs, h2T8_0)

        wfs = [load_wf(0), load_wf(1)]
        for h in range(4):
            emit_attn(h, 1)
        for m in range(16):
            if m + 2 < 32:
                wfs.append(load_wf(m + 2))
            emit_fc1(m, h2T8_0, gT8_0, *wfs[m])
        for h in range(4, 8):
            emit_attn(h, 1)
        for m in range(16, 32):
            if m + 2 < 32:
                wfs.append(load_wf(m + 2))
            emit_fc1(m, h2T8_0, gT8_0, *wfs[m])
        for h in range(8, 12):
            emit_attn(h, 1)
        w2s = [load_w2(0), load_w2(1)]
        fevs = []
        for m in range(4):
            if m + 2 < 8:
                w2s.append(load_w2(m + 2))
            fevs.append(emit_fc2(m, gT8_0, *w2s[m]))
        for h in range(12, H):
            emit_attn(h, 1)
        for m in range(4, 8):
            if m + 2 < 8:
                w2s.append(load_w2(m + 2))
            fevs.append(emit_fc2(m, gT8_0, *w2s[m]))
        for j in range(4):
            emit_out(j, xts[j][:], fevs)
        kqv_pool.release()

        # ---------------- Phase D: half 1 MLP ----------------
        h2T8_1 = h2_pool.tile([128, 8, 2, 512], F8, tag="h2T8")
        gT8_1 = g_pool.tile([128, 32, 2, 512], F8, tag="gT8")
        pevs = [emit_proj(m, 1) for m in range(8)]
        yT_pool.release()
        for j in range(4, 8):
            emit_x1_ln2(j, xts[j][:], pevs, h2T8_1)
        wfs = [load_wf(0), load_wf(1)]
        for m in range(32):
            if m + 2 < 32:
                wfs.append(load_wf(m + 2))
            emit_fc1(m, h2T8_1, gT8_1, *wfs[m])
        w2s = [load_w2(0), load_w2(1)]
        fevs = []
        for m in range(8):
            if m + 2 < 8:
                w2s.append(load_w2(m + 2))
            fevs.append(emit_fc2(m, gT8_1, *w2s[m]))
        for j in range(4, 8):
            emit_out(j, xts[j][:], fevs)

        for pool in (w2_pool, wf_pool, stg_pool, gf_pool, ev_pool, g_pool,
                     h2_pool, ytmp_pool, rbb_pool, rec_pool, et_pool, hb_pool,
                     s1s, wp_pool, xown, persist,
                     pt_pool, ya_pool, sc_pool, acc_pool):
            pool.release()

    nc.compile()
    return nc


_NC = None


def _q8(x, scale):
    return (np.asarray(x, np.float32) * scale).astype(NP8)


def _host_prepare(x, ln1_w, ln1_b, w_attn, b_attn, w_proj, b_proj,
                  ln2_w, ln2_b, w_fc, b_fc, w_fc2, b_fc2):
    f32 = np.float32
    ln1_w = np.asarray(ln1_w, f32); ln1_b = np.asarray(ln1_b, f32)
    w_attn = np.asarray(w_attn, f32); b_attn = np.asarray(b_attn, f32)
    scl = np.ones((3 * C,), f32)
    scl[:C] = 0.125
    w_full = ln1_w[:, None] * w_attn * scl[None, :]
    b_full = (ln1_b @ w_attn + b_attn) * scl

    # quad column permutation: slab s=(qd,j), col = lane*32+r -> (4qd+lane)*64+j*32+r
    qcol = np.zeros((8, 128), np.int64)
    for qd in range(4):
        for j in range(2):
            for lane in range(4):
                for r in range(32):
                    qcol[qd * 2 + j, lane * 32 + r] = (4 * qd + lane) * 64 + j * 32 + r

    def slab_qk(w, b, s_out):
        # w [C, C] -> [8, 128, 4, 2, 128], b -> [8, 128]
        ws = np.zeros((8, 128, 4, 2, 128), f32)
        bs = np.zeros((8, 128), f32)
        for s in range(8):
            wsel = w[:, qcol[s]] * S_W          # [C, 128]
            ws[s] = wsel.reshape(4, 2, 128, 128).transpose(2, 0, 1, 3)
            bs[s] = b[qcol[s]] * s_out
        return _q8(ws, 1.0), bs * 1.0

    wq8, bq = slab_qk(w_full[:, :C], b_full[:C], S_Q)
    wk8, bk = slab_qk(w_full[:, C:2 * C], b_full[C:2 * C], S_K)
    bqk = np.stack([bk, bq])    # [2, 8, 128]: [0]=k, [1]=q

    wv8 = _q8(w_full[:, 2 * C:].reshape(4, 2, 128, C).transpose(2, 0, 1, 3), S_W)
    bv8 = _q8(b_full[2 * C:].reshape(1, C), S_H * S_W)

    wp = np.asarray(w_proj, f32)
    wp8 = _q8(wp.reshape(4, 2, 128, 8, 128).transpose(3, 2, 0, 1, 4), S_W)
    bproj = np.asarray(b_proj, f32).reshape(8, 128)

    wf_eff = np.asarray(ln2_w, f32)[:, None] * np.asarray(w_fc, f32)
    bfc = (np.asarray(ln2_b, f32) @ np.asarray(w_fc, f32) +
           np.asarray(b_fc, f32)).reshape(32, 128)
    wfs = (wf_eff * S_W).reshape(4, 2, 128, 32, 128).transpose(3, 2, 0, 1, 4)
    wf8hi = wfs.astype(NP8)
    wf8lo = (wfs - wf8hi.astype(f32)).astype(NP8)

    w2 = np.asarray(w_fc2, f32)
    w2s = (w2 * S_W).reshape(16, 2, 128, 8, 128).transpose(3, 2, 0, 1, 4)
    wf28hi = w2s.astype(NP8)
    wf28lo = (w2s - wf28hi.astype(f32)).astype(NP8)
    bfc2 = np.asarray(b_fc2, f32).reshape(8, 128)

    shared = {
        "wq8": wq8, "wk8": wk8, "bqk": bqk, "wv8": wv8, "bv8": bv8,
        "wp8": wp8, "bproj": bproj,
        "wf8hi": wf8hi, "wf8lo": wf8lo, "bfc": bfc,
        "wf28hi": wf28hi, "wf28lo": wf28lo, "bfc2": bfc2,
    }
    # masks[p]: [128, 2, 128] fp8 multiplicative: [.,0,.]=lower-tri, [.,1,.]=parity
    ki = np.arange(128)[:, None]
    qi = np.arange(128)[None, :]
    diag = (ki <= qi).astype(f32)
    masks = []
    for p in range(2):
        m2 = np.full((128, 128), 1.0 if p == 1 else 0.0, f32)
        masks.append(np.stack([diag, m2], axis=1).astype(NP8))
    return shared, masks


def kernel(**inputs):
    global _NC
    if _NC is None:
        _NC = build_program()
    nc = _NC
    x = np.asarray(inputs["x"], np.float32)
    shared, masks = _host_prepare(**inputs)
    in_maps = []
    for c in range(8):
        b, p = c // 2, c % 2
        perm = [2 * j + p for j in range(8)] + [2 * j + 1 - p for j in range(8)]
        xp = np.ascontiguousarray(x[b].reshape(NCH, 128, C)[perm].reshape(T, C))
        im = dict(shared)
        im["x"] = xp
        im["masks"] = masks[p]
        in_maps.append(im)
    res = run_bass_kernel_spmd(nc, in_maps, list(range(8)), trace=False).results
    out = np.empty((B, T, C), np.float32)
    for c in range(8):
        b, p = c // 2, c % 2
        oc = res[c]["out"].reshape(NOWN, 128, C)
        for j in range(NOWN):
            out[b, (2 * j + p) * 128:(2 * j + p + 1) * 128, :] = oc[j]
    return out
